# revision 12
# baseline (speedup 1.0000x reference)
"""Causal self-attention (B=2, T=2048, D=1024, H=16) on 8 Trainium2 cores.

Sharding: tensor-parallel over heads x data-parallel over batch.
Core c handles batch b = c // 4 and the 4 heads hg = c % 4 (global heads
4*hg .. 4*hg+3).  Each core computes a partial output (its heads' slice of
the output projection); the host sums the 4 partials per batch and adds the
bias terms (Wp @ bv + bp), which commute with the attention average.

Device-side layout: everything is computed transposed so that no on-device
transposes are needed:
  qT/kT  [m, t]   (m = local head dim, on partitions), fp16
  v      [t, m]   (t on partitions) with a ones column appended per head, fp16
  sT     [t_k, t_q] = kT_h x qT_h -> exp (fp16) -> causal mask (affine_select)
  rawT   [65, t_q] = v_aug.T @ exp in fp32 PSUM (row 64 = softmax denominator)
  ctxT   [64, t_q] = rawT[:64] * (1/denom), denom broadcast on GPSIMD
  outT   [i, t]   partial projection in fp32, summed/transposed on host

Matmul operands are fp16 (PE runs fp32 matmuls at 1/4 rate); all
accumulation stays in fp32 PSUM.  Scores for the two heads of a partition
pair are issued back-to-back on PE row groups 0-63 / 64-127 so the 64-deep
PE window can run them concurrently (K=64 each).
"""

import sys

sys.path.insert(0, "/opt/trn_rl_repo")

import numpy as np

P = 128
B, T, D = 2, 2048, 1024
HD = 64
M = 256          # local q/k/v dim per core (4 heads x 64)
NH = 4           # local heads
DT = D // P      # 8 d tiles
NT = T // P      # 16 t_k tiles
TB = 512         # t_q block
NB = T // TB     # 4 t_q blocks
MT = M // P      # 2 m tiles
IT = D // P      # 8 output i tiles

N_CORES = 8

_CACHE = {}


def _build_nc(reps: int = 1, loop_n: int = 0):
    import contextlib

    import concourse.mybir as mybir
    from concourse import bacc
    from concourse.tile import TileContext

    fp32 = mybir.dt.float32
    fp16 = mybir.dt.float16
    AF = mybir.ActivationFunctionType

    nc = bacc.Bacc("TRN2", target_bir_lowering=False, debug=False)

    xT = nc.declare_dram_parameter("xT", [P, DT, T], fp16, isOutput=False)
    wqT = nc.declare_dram_parameter("wqT", [P, DT, M], fp16, isOutput=False)
    wkT = nc.declare_dram_parameter("wkT", [P, DT, M], fp16, isOutput=False)
    wvT = nc.declare_dram_parameter("wvT", [P, DT, M], fp16, isOutput=False)
    wpT = nc.declare_dram_parameter("wpT", [P, MT, D], fp16, isOutput=False)
    bq = nc.declare_dram_parameter("bq", [P, MT], fp32, isOutput=False)
    bk = nc.declare_dram_parameter("bk", [P, MT], fp32, isOutput=False)
    outT = nc.declare_dram_parameter("outT", [D, T], fp16, isOutput=True)

    scale = 1.0 / np.sqrt(HD)

    with TileContext(nc) as tc:
        with (
            tc.tile_pool(name="wconst", bufs=1) as wpool,
            tc.tile_pool(name="xbuf", bufs=1) as xpool,
            tc.tile_pool(name="qkv", bufs=1) as qpool,
            tc.tile_pool(name="exps", bufs=8) as spool,
            tc.tile_pool(name="small", bufs=4) as rpool,
            tc.tile_pool(name="psmm", bufs=2, space="PSUM") as psmm,
            tc.tile_pool(name="psacc", bufs=4, space="PSUM") as psacc,
        ):
            # ---- constants ----
            wq_sb = wpool.tile([P, DT, M], fp16, tag="wq")
            wk_sb = wpool.tile([P, DT, M], fp16, tag="wk")
            wv_sb = wpool.tile([P, DT, M], fp16, tag="wv")
            wp_sb = wpool.tile([P, MT, D], fp16, tag="wp")
            bq_sb = wpool.tile([P, MT], fp32, tag="bq")
            bk_sb = wpool.tile([P, MT], fp32, tag="bk")

            nc.sync.dma_start(wq_sb[:], wqT.ap())
            nc.sync.dma_start(wk_sb[:], wkT.ap())
            nc.sync.dma_start(wv_sb[:], wvT.ap())
            nc.sync.dma_start(wp_sb[:], wpT.ap())
            nc.sync.dma_start(bq_sb[:], bq.ap())
            nc.sync.dma_start(bk_sb[:], bk.ap())

            x_sb = xpool.tile([P, DT, T], fp16, tag="x")
            for dt in range(DT):
                nc.sync.dma_start(x_sb[:, dt, :], xT.ap()[:, dt, :])

            if loop_n:
                loop_cm = tc.For_i(
                    0, loop_n, 1,
                    hint_engines=(
                        mybir.EngineType.PE,
                        mybir.EngineType.Activation,
                        mybir.EngineType.DVE,
                        mybir.EngineType.Pool,
                        mybir.EngineType.SP,
                    ),
                )
            else:
                loop_cm = contextlib.nullcontext()
            with loop_cm:
              for _ in range(reps):
                qT_sb = qpool.tile([P, MT, T], fp16, tag="qT")
                kT_sb = qpool.tile([P, MT, T], fp16, tag="kT")
                # v with ones column: [t_part, tk, head, 65]
                va_sb = qpool.tile([P, NT, NH, HD + 1], fp16, tag="va")
                ca_sb = qpool.tile([P, MT, T], fp16, tag="ca")

                nc.vector.memset(va_sb[:, :, :, HD : HD + 1], 1.0)

                for tb in range(NB):
                    # ---- v tiles 4*tb .. 4*tb+3 (needed by this block) ----
                    for tq in range(2):
                        ps = psmm.tile([P, 2 * TB], fp32, tag="mm", name="psv")
                        for q2 in range(2):
                            tt = 4 * tb + 2 * tq + q2
                            for dt in range(DT):
                                nc.tensor.matmul(
                                    ps[:, q2 * M : (q2 + 1) * M],
                                    x_sb[:, dt, tt * P : (tt + 1) * P],
                                    wv_sb[:, dt, :],
                                    start=(dt == 0),
                                    stop=(dt == DT - 1),
                                )
                        nc.vector.tensor_copy(
                            va_sb[:, 4 * tb + 2 * tq : 4 * tb + 2 * tq + 2,
                                  :, 0:HD],
                            ps[:, : 2 * M].rearrange(
                                "p (t h d) -> p t h d", t=2, h=NH),
                        )

                    # ---- k^T and q^T for this t_q block ----
                    for w_sb, b_sb, dst in (
                        (wk_sb, bk_sb, kT_sb), (wq_sb, bq_sb, qT_sb),
                    ):
                        ps = psmm.tile([P, 2 * TB], fp32, tag="mm", name="psqk")
                        for j in range(MT):
                            for dt in range(DT):
                                nc.tensor.matmul(
                                    ps[:, j * TB : (j + 1) * TB],
                                    w_sb[:, dt, j * P : (j + 1) * P],
                                    x_sb[:, dt, tb * TB : (tb + 1) * TB],
                                    start=(dt == 0),
                                    stop=(dt == DT - 1),
                                )
                        for j in range(MT):
                            nc.vector.tensor_scalar_add(
                                dst[:, j, tb * TB : (tb + 1) * TB],
                                ps[:, j * TB : (j + 1) * TB],
                                b_sb[:, j : j + 1],
                            )

                    # ---- attention: heads paired on PE row groups ----
                    ntk = 4 * (tb + 1)  # causal: t_k tiles 0 .. 4*tb+3
                    for hp in range(NH // 2):
                        accs = [
                            psacc.tile([P, TB], fp32, tag="acc", name=f"acc{u}")
                            for u in range(2)
                        ]
                        for tt in range(ntk):
                            # both heads' scores in one 2-bank PSUM tile
                            ps_s = psmm.tile([P, 2 * TB], fp32, tag="mm",
                                             name="pss")
                            exp_t = spool.tile([P, 2 * TB], fp16, tag="exp")
                            di = tt - 4 * tb
                            for u in range(2):
                                pp = u * HD
                                nc.tensor.matmul(
                                    ps_s[:, u * TB : (u + 1) * TB],
                                    kT_sb[pp : pp + HD, hp,
                                          tt * P : (tt + 1) * P],
                                    qT_sb[pp : pp + HD, hp,
                                          tb * TB : (tb + 1) * TB],
                                    start=True,
                                    stop=True,
                                )
                            if di < 0:
                                nc.scalar.activation(
                                    exp_t[:], ps_s[:], AF.Exp, scale=scale
                                )
                            else:
                                c0 = di * P
                                for u in range(2):
                                    nc.scalar.activation(
                                        exp_t[:, u * TB + c0 : (u + 1) * TB],
                                        ps_s[:, u * TB + c0 : (u + 1) * TB],
                                        AF.Exp, scale=scale,
                                    )
                                    # keep iff (free - part - c0) >= 0; also
                                    # zero-fills the masked-off left region
                                    nc.gpsimd.affine_select(
                                        out=exp_t[:, u * TB : (u + 1) * TB],
                                        in_=exp_t[:, u * TB : (u + 1) * TB],
                                        compare_op=mybir.AluOpType.is_ge,
                                        fill=0.0,
                                        base=-c0,
                                        pattern=[[1, TB]],
                                        channel_multiplier=-1,
                                    )
                            for u in range(2):
                                nc.tensor.matmul(
                                    accs[u][: HD + 1, :],
                                    va_sb[:, tt, 2 * hp + u, :],
                                    exp_t[:, u * TB : (u + 1) * TB],
                                    start=(tt == 0),
                                    stop=(tt == ntk - 1),
                                )
                        # normalize: ctxT = raw * (1/denom)
                        for u in range(2):
                            rec = rpool.tile([1, TB], fp32, tag="rec")
                            nc.vector.reciprocal(rec[:], accs[u][HD : HD + 1, :])
                            rec_b = rpool.tile([HD, TB], fp32, tag="recb")
                            nc.gpsimd.partition_broadcast(rec_b[:], rec[:])
                            nc.vector.tensor_mul(
                                ca_sb[u * HD : (u + 1) * HD, hp,
                                      tb * TB : (tb + 1) * TB],
                                accs[u][0:HD, :],
                                rec_b[:],
                            )

                    # ---- output projection for this t_q block ----
                    for ip in range(IT // 2):
                        ps_o = psmm.tile([P, 2 * TB], fp32, tag="mm",
                                         name="pso")
                        for half in range(2):
                            it = 2 * ip + half
                            for jj in range(MT):
                                nc.tensor.matmul(
                                    ps_o[:, half * TB : (half + 1) * TB],
                                    wp_sb[:, jj, it * P : (it + 1) * P],
                                    ca_sb[:, jj, tb * TB : (tb + 1) * TB],
                                    start=(jj == 0),
                                    stop=(jj == MT - 1),
                                )
                        ot = spool.tile([P, 2 * TB], fp16, tag="ot")
                        nc.vector.tensor_copy(ot[:], ps_o[:])
                        # two 128-row output stripes share one SBUF tile
                        for half in range(2):
                            it = 2 * ip + half
                            nc.sync.dma_start(
                                outT.ap()[
                                    it * P : (it + 1) * P,
                                    tb * TB : (tb + 1) * TB,
                                ],
                                ot[:, half * TB : (half + 1) * TB],
                            )

    nc.finalize()
    return nc


def _prep_core_inputs(x, Wq, bq, Wk, bk, Wv, bv, Wp, bp, core):
    b = core // 4
    hg = core % 4
    sl = slice(hg * M, (hg + 1) * M)

    def part_inner(a2d):  # [D, F] -> [P, D//P, F], cast to fp16
        a = a2d.reshape(a2d.shape[0] // P, P, a2d.shape[1]).transpose(1, 0, 2)
        return np.ascontiguousarray(a.astype(np.float16))

    return {
        "xT": part_inner(x[b].T),                       # [128, 8, 2048]
        "wqT": part_inner(Wq[sl].T),                    # [128, 8, 256]
        "wkT": part_inner(Wk[sl].T),
        "wvT": part_inner(Wv[sl].T),
        "wpT": part_inner(Wp[:, sl].T),                 # [128, 2, 1024]
        "bq": np.ascontiguousarray(bq[sl].reshape(MT, P).T),
        "bk": np.ascontiguousarray(bk[sl].reshape(MT, P).T),
    }


def kernel(x, Wq, bq, Wk, bk, Wv, bv, Wp, bp):
    x = np.asarray(x, dtype=np.float32)
    Wq = np.asarray(Wq, dtype=np.float32)
    Wk = np.asarray(Wk, dtype=np.float32)
    Wv = np.asarray(Wv, dtype=np.float32)
    Wp = np.asarray(Wp, dtype=np.float32)
    bq = np.asarray(bq, dtype=np.float32)
    bk = np.asarray(bk, dtype=np.float32)
    bv = np.asarray(bv, dtype=np.float32)
    bp = np.asarray(bp, dtype=np.float32)

    if "nc" not in _CACHE:
        _CACHE["nc"] = _build_nc()
    nc = _CACHE["nc"]

    from concourse.bass_utils import run_bass_kernel_spmd

    in_maps = [
        _prep_core_inputs(x, Wq, bq, Wk, bk, Wv, bv, Wp, bp, c)
        for c in range(N_CORES)
    ]
    res = run_bass_kernel_spmd(nc, in_maps, list(range(N_CORES)))

    # v-bias and proj-bias commute with the softmax average:
    #   out = (probs @ (v + bv)) @ Wp.T + bp = raw_out + (Wp @ bv + bp)
    bias_vec = (Wp @ bv + bp).astype(np.float32)
    out = np.zeros((B, T, D), dtype=np.float32)
    for c in range(N_CORES):
        out[c // 4] += res.results[c]["outT"].T.astype(np.float32)
    out += bias_vec[None, None, :]
    return out


# revision 13
# speedup vs baseline: 1.1799x; 1.1799x over previous
"""Causal self-attention (B=2, T=2048, D=1024, H=16) on 8 Trainium2 cores.

Sharding: tensor-parallel over heads x data-parallel over batch.
Core c handles batch b = c // 4 and the 4 heads hg = c % 4 (global heads
4*hg .. 4*hg+3).  Each core computes a partial output (its heads' slice of
the output projection); the host sums the 4 partials per batch and adds the
bias terms (Wp @ bv + bp), which commute with the attention average.

Device-side layout: everything is computed transposed so that no on-device
transposes are needed:
  qT/kT  [m, t]   (m = local head dim, on partitions), fp16
  v      [t, m]   (t on partitions) with a ones column appended per head, fp16
  sT     [t_k, t_q] = kT_h x qT_h -> exp (fp16) -> causal mask (affine_select)
  rawT   [65, t_q] = v_aug.T @ exp in fp32 PSUM (row 64 = softmax denominator)
  ctxT   [64, t_q] = rawT[:64] * (1/denom), denom broadcast on GPSIMD
  outT   [i, t]   partial projection in fp32, summed/transposed on host

Matmul operands are fp16 (PE runs fp32 matmuls at 1/4 rate); all
accumulation stays in fp32 PSUM.  Scores for the two heads of a partition
pair are issued back-to-back on PE row groups 0-63 / 64-127 so the 64-deep
PE window can run them concurrently (K=64 each).
"""

import sys

sys.path.insert(0, "/opt/trn_rl_repo")

import numpy as np

P = 128
B, T, D = 2, 2048, 1024
HD = 64
M = 256          # local q/k/v dim per core (4 heads x 64)
NH = 4           # local heads
DT = D // P      # 8 d tiles
NT = T // P      # 16 t_k tiles
TB = 512         # t_q block
NB = T // TB     # 4 t_q blocks
MT = M // P      # 2 m tiles
IT = D // P      # 8 output i tiles

N_CORES = 8

_CACHE = {}


def _build_nc(reps: int = 1, loop_n: int = 0):
    import contextlib

    import concourse.mybir as mybir
    from concourse import bacc
    from concourse.tile import TileContext

    fp32 = mybir.dt.float32
    fp16 = mybir.dt.float16
    AF = mybir.ActivationFunctionType

    nc = bacc.Bacc("TRN2", target_bir_lowering=False, debug=False)

    xT = nc.declare_dram_parameter("xT", [P, DT, T], fp16, isOutput=False)
    wqT = nc.declare_dram_parameter("wqT", [P, DT, M], fp16, isOutput=False)
    wkT = nc.declare_dram_parameter("wkT", [P, DT, M], fp16, isOutput=False)
    wvT = nc.declare_dram_parameter("wvT", [P, DT, M], fp16, isOutput=False)
    wpT = nc.declare_dram_parameter("wpT", [P, MT, D], fp16, isOutput=False)
    bq = nc.declare_dram_parameter("bq", [P, MT], fp32, isOutput=False)
    bk = nc.declare_dram_parameter("bk", [P, MT], fp32, isOutput=False)
    outT = nc.declare_dram_parameter("outT", [D, T], fp16, isOutput=True)

    scale = 1.0 / np.sqrt(HD)

    with TileContext(nc) as tc:
        with (
            tc.tile_pool(name="wconst", bufs=1) as wpool,
            tc.tile_pool(name="xbuf", bufs=1) as xpool,
            tc.tile_pool(name="qkv", bufs=1) as qpool,
            tc.tile_pool(name="exps", bufs=8) as spool,
            tc.tile_pool(name="small", bufs=4) as rpool,
            tc.tile_pool(name="psqkv", bufs=2, space="PSUM") as psqkv,
            tc.tile_pool(name="psatt", bufs=4, space="PSUM") as psatt,
            tc.tile_pool(name="psacc", bufs=2, space="PSUM") as psacc,
        ):
            # ---- constants ----
            wq_sb = wpool.tile([P, DT, M], fp16, tag="wq")
            wk_sb = wpool.tile([P, DT, M], fp16, tag="wk")
            wv_sb = wpool.tile([P, DT, M], fp16, tag="wv")
            wp_sb = wpool.tile([P, MT, D], fp16, tag="wp")
            bq_sb = wpool.tile([P, MT], fp32, tag="bq")
            bk_sb = wpool.tile([P, MT], fp32, tag="bk")

            nc.sync.dma_start(wq_sb[:], wqT.ap())
            nc.sync.dma_start(wk_sb[:], wkT.ap())
            nc.sync.dma_start(wv_sb[:], wvT.ap())
            nc.sync.dma_start(wp_sb[:], wpT.ap())
            nc.sync.dma_start(bq_sb[:], bq.ap())
            nc.sync.dma_start(bk_sb[:], bk.ap())

            x_sb = xpool.tile([P, DT, T], fp16, tag="x")
            for dt in range(DT):
                nc.sync.dma_start(x_sb[:, dt, :], xT.ap()[:, dt, :])

            if loop_n:
                loop_cm = tc.For_i(
                    0, loop_n, 1,
                    hint_engines=(
                        mybir.EngineType.PE,
                        mybir.EngineType.Activation,
                        mybir.EngineType.DVE,
                        mybir.EngineType.Pool,
                        mybir.EngineType.SP,
                    ),
                )
            else:
                loop_cm = contextlib.nullcontext()
            with loop_cm:
              for _ in range(reps):
                qT_sb = qpool.tile([P, MT, T], fp16, tag="qT")
                kT_sb = qpool.tile([P, MT, T], fp16, tag="kT")
                # v with ones column: [t_part, tk, head, 65]
                va_sb = qpool.tile([P, NT, NH, HD + 1], fp16, tag="va")
                ca_sb = qpool.tile([P, MT, T], fp16, tag="ca")

                nc.vector.memset(va_sb[:, :, :, HD : HD + 1], 1.0)

                def emit_proj(tb):
                    for it in range(IT):
                        ps_o = psqkv.tile([P, TB], fp32, tag="mm", name="pso")
                        for jj in range(MT):
                            nc.tensor.matmul(
                                ps_o[:],
                                wp_sb[:, jj, it * P : (it + 1) * P],
                                ca_sb[:, jj, tb * TB : (tb + 1) * TB],
                                start=(jj == 0),
                                stop=(jj == MT - 1),
                            )
                        ot = spool.tile([P, TB], fp16, tag="ot")
                        nc.vector.tensor_copy(ot[:], ps_o[:])
                        nc.sync.dma_start(
                            outT.ap()[it * P : (it + 1) * P,
                                      tb * TB : (tb + 1) * TB],
                            ot[:],
                        )

                for tb in range(NB):
                    # ---- v tiles 4*tb .. 4*tb+3 ----
                    for tq in range(2):
                        ps = psqkv.tile([P, TB], fp32, tag="mm", name="psv")
                        for q2 in range(2):
                            tt = 4 * tb + 2 * tq + q2
                            for dt in range(DT):
                                nc.tensor.matmul(
                                    ps[:, q2 * M : (q2 + 1) * M],
                                    x_sb[:, dt, tt * P : (tt + 1) * P],
                                    wv_sb[:, dt, :],
                                    start=(dt == 0),
                                    stop=(dt == DT - 1),
                                )
                        nc.vector.tensor_copy(
                            va_sb[:, 4 * tb + 2 * tq : 4 * tb + 2 * tq + 2,
                                  :, 0:HD],
                            ps[:].rearrange("p (t h d) -> p t h d", t=2, h=NH),
                        )

                    # ---- k^T and q^T for this t_q block ----
                    for w_sb, b_sb, dst in (
                        (wk_sb, bk_sb, kT_sb), (wq_sb, bq_sb, qT_sb),
                    ):
                        for j in range(MT):
                            ps = psqkv.tile([P, TB], fp32, tag="mm",
                                            name="psqk")
                            for dt in range(DT):
                                nc.tensor.matmul(
                                    ps[:],
                                    w_sb[:, dt, j * P : (j + 1) * P],
                                    x_sb[:, dt, tb * TB : (tb + 1) * TB],
                                    start=(dt == 0),
                                    stop=(dt == DT - 1),
                                )
                            nc.vector.tensor_scalar_add(
                                dst[:, j, tb * TB : (tb + 1) * TB],
                                ps[:],
                                b_sb[:, j : j + 1],
                            )

                    # ---- attention, software-pipelined so the in-order PE
                    # issues score matmuls OFF steps ahead of the PV matmuls
                    # that consume the exp results ----
                    ntk = 4 * (tb + 1)  # causal: t_k tiles 0 .. 4*tb+3
                    OFF = 2
                    for hp in range(NH // 2):
                        accs = [
                            psacc.tile([P, TB], fp32, tag="acc", name=f"acc{u}")
                            for u in range(2)
                        ]
                        pend = []  # (tt, exp_t)

                        def emit_pv(hp=hp, accs=accs, ntk=ntk):
                            tt, exp_t = pend.pop(0)
                            for u in range(2):
                                nc.tensor.matmul(
                                    accs[u][: HD + 1, :],
                                    va_sb[:, tt, 2 * hp + u, :],
                                    exp_t[:, u * TB : (u + 1) * TB],
                                    start=(tt == 0),
                                    stop=(tt == ntk - 1),
                                )

                        for tt in range(ntk):
                            exp_t = spool.tile([P, 2 * TB], fp16, tag="exp")
                            di = tt - 4 * tb
                            for u in range(2):
                                ps_s = psatt.tile([P, TB], fp32, tag="ss",
                                                  name=f"ss{u}")
                                pp = u * HD
                                nc.tensor.matmul(
                                    ps_s[:],
                                    kT_sb[pp : pp + HD, hp,
                                          tt * P : (tt + 1) * P],
                                    qT_sb[pp : pp + HD, hp,
                                          tb * TB : (tb + 1) * TB],
                                    start=True,
                                    stop=True,
                                )
                                if di < 0:
                                    nc.scalar.activation(
                                        exp_t[:, u * TB : (u + 1) * TB],
                                        ps_s[:], AF.Exp, scale=scale,
                                    )
                                else:
                                    c0 = di * P
                                    nc.scalar.activation(
                                        exp_t[:, u * TB + c0 : (u + 1) * TB],
                                        ps_s[:, c0:], AF.Exp, scale=scale,
                                    )
                                    # keep iff (free - part - c0) >= 0; also
                                    # zero-fills the masked-off left region
                                    nc.gpsimd.affine_select(
                                        out=exp_t[:, u * TB : (u + 1) * TB],
                                        in_=exp_t[:, u * TB : (u + 1) * TB],
                                        compare_op=mybir.AluOpType.is_ge,
                                        fill=0.0,
                                        base=-c0,
                                        pattern=[[1, TB]],
                                        channel_multiplier=-1,
                                    )
                            pend.append((tt, exp_t))
                            if len(pend) > OFF:
                                emit_pv()
                        while pend:
                            emit_pv()
                        # normalize: ctxT = raw * (1/denom)
                        for u in range(2):
                            rec = rpool.tile([1, TB], fp32, tag="rec")
                            nc.vector.reciprocal(rec[:], accs[u][HD : HD + 1, :])
                            rec_b = rpool.tile([HD, TB], fp32, tag="recb")
                            nc.gpsimd.partition_broadcast(rec_b[:], rec[:])
                            nc.vector.tensor_mul(
                                ca_sb[u * HD : (u + 1) * HD, hp,
                                      tb * TB : (tb + 1) * TB],
                                accs[u][0:HD, :],
                                rec_b[:],
                            )

                    # proj for the previous block fills PE while this block's
                    # normalize chain drains
                    if tb > 0:
                        emit_proj(tb - 1)
                emit_proj(NB - 1)

    nc.finalize()
    return nc


def _prep_core_inputs(x, Wq, bq, Wk, bk, Wv, bv, Wp, bp, core):
    b = core // 4
    hg = core % 4
    sl = slice(hg * M, (hg + 1) * M)

    def part_inner(a2d):  # [D, F] -> [P, D//P, F], cast to fp16
        a = a2d.reshape(a2d.shape[0] // P, P, a2d.shape[1]).transpose(1, 0, 2)
        return np.ascontiguousarray(a.astype(np.float16))

    return {
        "xT": part_inner(x[b].T),                       # [128, 8, 2048]
        "wqT": part_inner(Wq[sl].T),                    # [128, 8, 256]
        "wkT": part_inner(Wk[sl].T),
        "wvT": part_inner(Wv[sl].T),
        "wpT": part_inner(Wp[:, sl].T),                 # [128, 2, 1024]
        "bq": np.ascontiguousarray(bq[sl].reshape(MT, P).T),
        "bk": np.ascontiguousarray(bk[sl].reshape(MT, P).T),
    }


def kernel(x, Wq, bq, Wk, bk, Wv, bv, Wp, bp):
    x = np.asarray(x, dtype=np.float32)
    Wq = np.asarray(Wq, dtype=np.float32)
    Wk = np.asarray(Wk, dtype=np.float32)
    Wv = np.asarray(Wv, dtype=np.float32)
    Wp = np.asarray(Wp, dtype=np.float32)
    bq = np.asarray(bq, dtype=np.float32)
    bk = np.asarray(bk, dtype=np.float32)
    bv = np.asarray(bv, dtype=np.float32)
    bp = np.asarray(bp, dtype=np.float32)

    if "nc" not in _CACHE:
        _CACHE["nc"] = _build_nc()
    nc = _CACHE["nc"]

    from concourse.bass_utils import run_bass_kernel_spmd

    in_maps = [
        _prep_core_inputs(x, Wq, bq, Wk, bk, Wv, bv, Wp, bp, c)
        for c in range(N_CORES)
    ]
    res = run_bass_kernel_spmd(nc, in_maps, list(range(N_CORES)))

    # v-bias and proj-bias commute with the softmax average:
    #   out = (probs @ (v + bv)) @ Wp.T + bp = raw_out + (Wp @ bv + bp)
    bias_vec = (Wp @ bv + bp).astype(np.float32)
    out = np.zeros((B, T, D), dtype=np.float32)
    for c in range(N_CORES):
        out[c // 4] += res.results[c]["outT"].T.astype(np.float32)
    out += bias_vec[None, None, :]
    return out
